# revision 34
# baseline (speedup 1.0000x reference)
"""BEV voxel-pooling (segment_reduce) kernel for 8 Trainium2 NeuronCores.

Strategy
--------
Host (numpy, cheap — driven only by the small geometry inputs):
  * compute each point's BEV rank (bin id) exactly as the reference does
  * per sample, group points by rank (segment); split each segment into
    pieces of <= LCAP points, padded to a power-of-2 length; single-point
    pieces never leave the host (their sum is the point itself)
  * sort pieces by length desc and deal them round-robin into 4 shards
    (x2 samples -> 8 cores), so every core sees a near-identical
    piece-length profile and one SPMD group schedule fits all cores
  * pack pieces point-major: an SBUF tile [128, G pieces, L, 64 ch] fp16
    per group of G*128 equal-length pieces (channel runs of 128 B)

Device (per core, one SPMD Bass/Tile program — DVE + DMA only):
  * per group: DMA the tile in; a log2(L)-deep fp16 tensor_tensor fold
    tree halves L in place on the vector engine (2 elem/cycle/lane,
    contiguous 128 B runs); the last fold writes [128, G*64] fp16 piece
    sums into a stage tile. Short-piece groups stream first (tiny
    transfers warm the pipeline), the bulk follows, a single-bucket
    group lands last so the post-stream tail is short.
  * three coalesced stage->DRAM output DMAs on the scalar engine's ring
    overlap the fold train; everything else overlaps the input stream,
    which runs at the per-core HBM roofline.

Host gather: piece sums (fp32 upconvert) + single-point rows ->
np.add.reduceat by (sample, rank) -> BEV grid.
"""
import sys
sys.path.insert(0, '/opt/trn_rl_repo')

import numpy as np

# ---------------- problem constants (hardcoded per spec) ----------------
B, N, C = 2, 6, 64
H_IMG, W_IMG = 256, 704
DS = 16
DSH, DSW = H_IMG // DS, W_IMG // DS          # 16, 44
D0, D1 = 4, 45                                # depth bins -> D = 41
X, Y, Z = 200, 200, 1
NBINS = X * Y * Z                             # 40000
NP_SAMPLE = N * (D1 - D0) * DSH * DSW         # 173184
NCORES = 8
SHARDS_PER_SAMPLE = 4
LCAP = 16                                     # max points per piece (pow2)

_compiled = {}


def _pow2ceil(x):
    return 1 << (int(x) - 1).bit_length()


# ---------------- host geometry (matches reference numerics) ----------------
def _compute_ranks(frustum, post_trans, post_rots, intrinsics, extrinsics,
                   bev_res, bev_start_pos):
    frustum = np.asarray(frustum, np.float32)
    post_trans = np.asarray(post_trans, np.float32)
    post_rots = np.asarray(post_rots, np.float32)
    intrinsics = np.asarray(intrinsics, np.float32)
    extrinsics = np.asarray(extrinsics, np.float32)
    bev_res = np.asarray(bev_res, np.float32)
    bev_start_pos = np.asarray(bev_start_pos, np.float32)

    ext_inv = np.linalg.inv(extrinsics.astype(np.float64)).astype(np.float32)
    rot = ext_inv[..., :3, :3]
    trans = ext_inv[..., :3, 3]
    pts = frustum[None, None] - post_trans[:, :, None, None, None, :]
    pr_inv = np.linalg.inv(post_rots.astype(np.float64)).astype(np.float32)
    pts = np.einsum('bnij,bndhwj->bndhwi', pr_inv, pts).astype(np.float32)
    pts = np.concatenate([pts[..., :2] * pts[..., 2:3], pts[..., 2:3]], axis=-1)
    comb = (rot @ np.linalg.inv(intrinsics.astype(np.float64)).astype(np.float32)
            ).astype(np.float32)
    pts = np.einsum('bnij,bndhwj->bndhwi', comb, pts).astype(np.float32)
    geom = pts + trans[:, :, None, None, None, :]

    coords = (geom - (bev_start_pos - bev_res / 2.0)) / bev_res
    ci = coords.reshape(B, -1, 3).astype(np.int32)
    mask = ((ci[..., 0] >= 0) & (ci[..., 0] < X) &
            (ci[..., 1] >= 0) & (ci[..., 1] < Y) &
            (ci[..., 2] >= 0) & (ci[..., 2] < Z))
    rank = ci[..., 0] * (Y * Z) + ci[..., 1] * Z + ci[..., 2]
    return rank, mask


# ---------------- host planning ----------------
class CorePlan:
    __slots__ = ("sample", "piece_start", "piece_len", "piece_rank", "order")


def _plan_cores(rank, mask):
    """Per-core piece lists, single-point host rows, shared group schedule.

    Returns (plans, host_singles, sched); sched is a tuple of
    (bucket_start, G, L) in device-emit order.
    """
    plans = []
    host_singles = []        # (sample, point_idx array, rank array)
    for b in range(B):
        r = rank[b]
        m = mask[b]
        valid_idx = np.nonzero(m)[0]
        order = valid_idx[np.argsort(r[valid_idx], kind='stable')]
        rs = r[order]
        newseg = np.r_[True, rs[1:] != rs[:-1]]
        seg_start = np.nonzero(newseg)[0]
        seg_len = np.diff(np.r_[seg_start, len(rs)])
        seg_rank = rs[seg_start]
        # split each segment into pieces of <= LCAP
        n_pieces = -(-seg_len // LCAP)
        pc_seg = np.repeat(np.arange(len(seg_len)), n_pieces)
        idx_in_seg = np.arange(len(pc_seg)) - np.repeat(
            np.cumsum(n_pieces) - n_pieces, n_pieces)
        pc_start = seg_start[pc_seg] + idx_in_seg * LCAP
        pc_len = np.minimum(seg_len[pc_seg] - idx_in_seg * LCAP, LCAP)
        pc_rank = seg_rank[pc_seg]
        # single-point pieces: no reduction needed, keep on host
        ones = pc_len == 1
        host_singles.append((b, order[pc_start[ones]], pc_rank[ones]))
        keep = ~ones
        pc_start, pc_len, pc_rank = pc_start[keep], pc_len[keep], pc_rank[keep]
        # length-desc sort, deal round-robin into shards
        srt = np.argsort(-pc_len, kind='stable')
        for s in range(SHARDS_PER_SAMPLE):
            sel = srt[s::SHARDS_PER_SAMPLE]
            pl = CorePlan()
            pl.sample = b
            pl.order = order
            pl.piece_start = pc_start[sel]
            pl.piece_len = pc_len[sel]
            pl.piece_rank = pc_rank[sel]
            plans.append(pl)

    nbuckets = max(-(-len(pl.piece_len) // 128) for pl in plans)
    bl = []
    for k in range(nbuckets):
        L = 2
        for pl in plans:
            if len(pl.piece_len) > k * 128:
                L = max(L, _pow2ceil(pl.piece_len[k * 128]))
        bl.append(L)

    big = [k for k in range(nbuckets) if bl[k] >= LCAP]
    small = [k for k in range(nbuckets) if bl[k] < LCAP]

    sched = []
    # small classes first (tiny transfers warm the DVE), ascending L,
    # grouped by equal L
    for k in sorted(small, key=lambda k: bl[k]):
        if (sched and sched[-1][2] == bl[k]
                and sched[-1][0] + sched[-1][1] == k and sched[-1][1] < 4):
            sched[-1] = (sched[-1][0], sched[-1][1] + 1, bl[k])
        else:
            sched.append((k, 1, bl[k]))
    # big groups: single-bucket head (lands first, starts the DVE fold
    # train early), big middle, single-bucket tail
    nbig = len(big)
    caps = [1, 2, 3] + [4] * max(0, (nbig - 7) // 4 + 1) + [3, 1]
    k = 0
    for cap in caps:
        if k >= nbig:
            break
        G = min(cap, nbig - k)
        if nbig - (k + G) == 0 and cap != 1 and G > 1:
            G -= 1          # always leave a single-bucket final group
        sched.append((big[k], G, LCAP))
        k += G
    while k < nbig:
        sched.append((big[k], 1, LCAP))
        k += 1
    return plans, host_singles, tuple(sched)


def _build_table(pl, feats16_b, sched):
    """Pack one core's pieces into the [128, sum(G*L*64)] fp16 table."""
    totc = sum(G * L * 64 for _, G, L in sched)
    table = np.zeros((128, totc), np.float16)
    off = 0
    np_pieces = len(pl.piece_len)
    for k0, G, L in sched:
        for b in range(G):
            lo = (k0 + b) * 128
            hi = min(lo + 128, np_pieces)
            if hi > lo:
                lens = pl.piece_len[lo:hi]
                starts = pl.piece_start[lo:hi]
                p_ids = np.repeat(np.arange(hi - lo), lens)
                j_ids = np.arange(len(p_ids)) - np.repeat(
                    np.cumsum(lens) - lens, lens)
                pts = pl.order[np.repeat(starts, lens) + j_ids]
                view = table[:, off + b * L * 64: off + (b + 1) * L * 64
                             ].reshape(128, L, 64)
                view[p_ids, j_ids, :] = feats16_b[pts]
        off += G * L * 64
    return table


# ---------------- device program ----------------
def _build_kernel(sched):
    import concourse.bass as bass
    import concourse.bacc as bacc
    import concourse.mybir as mybir
    import concourse.tile as tile
    from contextlib import ExitStack

    F16 = mybir.dt.float16
    totc = sum(G * L * 64 for _, G, L in sched)
    nbt = sum(G for _, G, L in sched)

    nc = bacc.Bacc()
    table = nc.dram_tensor("table", [128, totc], F16, kind="ExternalInput")
    out = nc.dram_tensor("out", [128, nbt * 64], F16, kind="ExternalOutput")

    with tile.TileContext(nc) as tc, ExitStack() as ctx:
        pool = ctx.enter_context(tc.tile_pool(name="bkt", bufs=1))
        stp = ctx.enter_context(tc.tile_pool(name="stage", bufs=1))
        stage = stp.tile([128, nbt * 64], F16)

        # leading small-L groups share one tile + one DMA issued from the
        # idle tensor engine's ring, so it streams in parallel with the
        # sync-ring big-group DMAs instead of serializing behind their
        # ~0.6 us-per-instruction descriptor writes
        nsmall = 0
        while nsmall < len(sched) and sched[nsmall][2] < LCAP:
            nsmall += 1
        small_cols = sum(G * L * 64 for _, G, L in sched[:nsmall])

        tiles = []
        off = 0
        for g, (_, G, L) in enumerate(sched):
            t = pool.tile([128, G * L * 64], F16, tag=f"g{g}")
            nc.sync.dma_start(t[:], table[:, off:off + G * L * 64])
            tiles.append(t[:])
            off += G * L * 64

        ngroups = len(sched)
        cut1 = max(1, int(ngroups * 0.5))
        cut2 = max(cut1 + 1, int(ngroups * 0.85))
        cut3 = max(cut2 + 1, ngroups - 1)
        boffs = []
        boff = 0
        for _, G, L in sched:
            boffs.append(boff)
            boff += G

        def fold(g):
            _, G, L = sched[g]
            b0 = boffs[g]
            v = tiles[g].rearrange("p (b l c) -> p b l c", l=L, c=64)
            st = stage[:, b0 * 64:(b0 + G) * 64].rearrange(
                "p (b o c) -> p b o c", o=1, c=64)
            cur = L
            while cur > 2:
                h = cur // 2
                nc.vector.tensor_tensor(
                    v[:, :, 0:h, :], v[:, :, 0:h, :], v[:, :, h:cur, :],
                    mybir.AluOpType.add)
                cur = h
            nc.vector.tensor_tensor(
                st, v[:, :, 0:1, :], v[:, :, 1:2, :], mybir.AluOpType.add)

        # groups stream smallest-first on the sync ring; fold in the same
        # order (the 64 KB L2 class lands first and starts the DVE early)
        order = list(range(ngroups))
        done = 0
        out_lo = 0
        emitted = set()
        for g in order:
            fold(g)
            emitted.add(g)
            done += 1
            if done in (cut1, cut2, cut3, ngroups):
                hi = 0
                while hi < ngroups and hi in emitted:
                    hi += 1
                hi_off = boffs[hi - 1] + sched[hi - 1][1] if hi else 0
                if hi and hi_off > out_lo:
                    nc.scalar.dma_start(
                        out[:, out_lo * 64:hi_off * 64],
                        stage[:, out_lo * 64:hi_off * 64])
                    out_lo = hi_off
    nc.finalize()
    return nc


# ---------------- entry point ----------------
def kernel(image_feature, post_trans, post_rots, intrinsics, extrinsics,
           frustum, bev_res, bev_start_pos):
    from concourse.bass_utils import run_bass_kernel_spmd
    import os

    rank, mask = _compute_ranks(frustum, post_trans, post_rots, intrinsics,
                                extrinsics, bev_res, bev_start_pos)
    feats16 = np.asarray(image_feature, np.float32).reshape(
        B, NP_SAMPLE, C).astype(np.float16)
    plans, host_singles, sched = _plan_cores(rank, mask)

    in_maps = [{"table": _build_table(pl, feats16[pl.sample], sched)}
               for pl in plans]

    if sched not in _compiled:
        _compiled[sched] = _build_kernel(sched)
    nc = _compiled[sched]

    trace = bool(int(os.environ.get("BEV_TRACE", "0")))
    res = run_bass_kernel_spmd(nc, in_maps, core_ids=list(range(NCORES)),
                               trace=trace,
                               trace_cores=[0] if trace else None)
    if trace and res.exec_time_ns is not None:
        print(f"HW exec time: {res.exec_time_ns} ns")
        kernel.last_exec_time_ns = res.exec_time_ns
        kernel.last_results = res

    nbt = sum(G for _, G, L in sched)
    bucket_seq = []
    for k0, G, L in sched:
        bucket_seq.extend(range(k0, k0 + G))
    keys = []
    rows = []
    for k, pl in enumerate(plans):
        o = res.results[k]["out"]                      # [128, nbt*64] f16
        r = o.reshape(128, nbt, 64).swapaxes(0, 1)     # [slot, p, 64]
        npieces = len(pl.piece_len)
        for slot, bk in enumerate(bucket_seq):
            lo = bk * 128
            hi = min(lo + 128, npieces)
            if hi > lo:
                keys.append(pl.sample * NBINS
                            + pl.piece_rank[lo:hi].astype(np.int64))
                rows.append(r[slot, :hi - lo])
    for b, pts, rks in host_singles:
        if len(pts):
            keys.append(b * NBINS + rks.astype(np.int64))
            rows.append(feats16[b][pts])
    keys = np.concatenate(keys)
    rows = np.concatenate(rows).astype(np.float32)
    srt = np.argsort(keys, kind='stable')
    ks = keys[srt]
    bounds = np.r_[0, np.nonzero(np.diff(ks))[0] + 1]
    sums = np.add.reduceat(rows[srt], bounds, axis=0)
    grid = np.zeros((B * NBINS, C), np.float32)
    grid[ks[bounds]] = sums
    return np.ascontiguousarray(
        grid.reshape(B, X, Y, C).transpose(0, 3, 1, 2))


# revision 35
# speedup vs baseline: 1.1178x; 1.1178x over previous
"""BEV voxel-pooling (segment_reduce) kernel for 8 Trainium2 NeuronCores.

Strategy
--------
Host (numpy, cheap — driven only by the small geometry inputs):
  * compute each point's BEV rank (bin id) exactly as the reference does
  * per sample, group points by rank (segment); split each segment into
    pieces of <= LCAP points, padded to a power-of-2 length; single-point
    pieces never leave the host (their sum is the point itself)
  * sort pieces by length desc and deal them round-robin into 4 shards
    (x2 samples -> 8 cores), so every core sees a near-identical
    piece-length profile and one SPMD group schedule fits all cores
  * pack pieces point-major: an SBUF tile [128, G pieces, L, 64 ch] fp16
    per group of G*128 equal-length pieces (channel runs of 128 B)

Device (per core, one SPMD Bass/Tile program — DVE + DMA only):
  * per group: DMA the tile in; a log2(L)-deep fp16 tensor_tensor fold
    tree halves L in place on the vector engine (2 elem/cycle/lane,
    contiguous 128 B runs); the last fold writes [128, G*64] fp16 piece
    sums into a stage tile. Short-piece groups stream first (tiny
    transfers warm the pipeline), the bulk follows, a single-bucket
    group lands last so the post-stream tail is short.
  * three coalesced stage->DRAM output DMAs on the scalar engine's ring
    overlap the fold train; everything else overlaps the input stream,
    which runs at the per-core HBM roofline.

Host gather: piece sums (fp32 upconvert) + single-point rows ->
np.add.reduceat by (sample, rank) -> BEV grid.
"""
import sys
sys.path.insert(0, '/opt/trn_rl_repo')

import numpy as np

# ---------------- problem constants (hardcoded per spec) ----------------
B, N, C = 2, 6, 64
H_IMG, W_IMG = 256, 704
DS = 16
DSH, DSW = H_IMG // DS, W_IMG // DS          # 16, 44
D0, D1 = 4, 45                                # depth bins -> D = 41
X, Y, Z = 200, 200, 1
NBINS = X * Y * Z                             # 40000
NP_SAMPLE = N * (D1 - D0) * DSH * DSW         # 173184
NCORES = 8
SHARDS_PER_SAMPLE = 4
LCAP = 16                                     # max points per piece (pow2)

_compiled = {}


def _pow2ceil(x):
    return 1 << (int(x) - 1).bit_length()


# ---------------- host geometry (matches reference numerics) ----------------
def _compute_ranks(frustum, post_trans, post_rots, intrinsics, extrinsics,
                   bev_res, bev_start_pos):
    frustum = np.asarray(frustum, np.float32)
    post_trans = np.asarray(post_trans, np.float32)
    post_rots = np.asarray(post_rots, np.float32)
    intrinsics = np.asarray(intrinsics, np.float32)
    extrinsics = np.asarray(extrinsics, np.float32)
    bev_res = np.asarray(bev_res, np.float32)
    bev_start_pos = np.asarray(bev_start_pos, np.float32)

    ext_inv = np.linalg.inv(extrinsics.astype(np.float64)).astype(np.float32)
    rot = ext_inv[..., :3, :3]
    trans = ext_inv[..., :3, 3]
    pts = frustum[None, None] - post_trans[:, :, None, None, None, :]
    pr_inv = np.linalg.inv(post_rots.astype(np.float64)).astype(np.float32)
    pts = np.einsum('bnij,bndhwj->bndhwi', pr_inv, pts).astype(np.float32)
    pts = np.concatenate([pts[..., :2] * pts[..., 2:3], pts[..., 2:3]], axis=-1)
    comb = (rot @ np.linalg.inv(intrinsics.astype(np.float64)).astype(np.float32)
            ).astype(np.float32)
    pts = np.einsum('bnij,bndhwj->bndhwi', comb, pts).astype(np.float32)
    geom = pts + trans[:, :, None, None, None, :]

    coords = (geom - (bev_start_pos - bev_res / 2.0)) / bev_res
    ci = coords.reshape(B, -1, 3).astype(np.int32)
    mask = ((ci[..., 0] >= 0) & (ci[..., 0] < X) &
            (ci[..., 1] >= 0) & (ci[..., 1] < Y) &
            (ci[..., 2] >= 0) & (ci[..., 2] < Z))
    rank = ci[..., 0] * (Y * Z) + ci[..., 1] * Z + ci[..., 2]
    return rank, mask


# ---------------- host planning ----------------
class CorePlan:
    __slots__ = ("sample", "piece_start", "piece_len", "piece_rank", "order")


def _plan_cores(rank, mask):
    """Per-core piece lists, single-point host rows, shared group schedule.

    Returns (plans, host_singles, sched); sched is a tuple of
    (bucket_start, G, L) in device-emit order.
    """
    plans = []
    host_singles = []        # (sample, point_idx array, rank array)
    for b in range(B):
        r = rank[b]
        m = mask[b]
        valid_idx = np.nonzero(m)[0]
        order = valid_idx[np.argsort(r[valid_idx], kind='stable')]
        rs = r[order]
        newseg = np.r_[True, rs[1:] != rs[:-1]]
        seg_start = np.nonzero(newseg)[0]
        seg_len = np.diff(np.r_[seg_start, len(rs)])
        seg_rank = rs[seg_start]
        # split each segment into pieces of <= LCAP
        n_pieces = -(-seg_len // LCAP)
        pc_seg = np.repeat(np.arange(len(seg_len)), n_pieces)
        idx_in_seg = np.arange(len(pc_seg)) - np.repeat(
            np.cumsum(n_pieces) - n_pieces, n_pieces)
        pc_start = seg_start[pc_seg] + idx_in_seg * LCAP
        pc_len = np.minimum(seg_len[pc_seg] - idx_in_seg * LCAP, LCAP)
        pc_rank = seg_rank[pc_seg]
        # single-point pieces: no reduction needed, keep on host
        ones = pc_len == 1
        host_singles.append((b, order[pc_start[ones]], pc_rank[ones]))
        keep = ~ones
        pc_start, pc_len, pc_rank = pc_start[keep], pc_len[keep], pc_rank[keep]
        # length-desc sort, deal round-robin into shards
        srt = np.argsort(-pc_len, kind='stable')
        for s in range(SHARDS_PER_SAMPLE):
            sel = srt[s::SHARDS_PER_SAMPLE]
            pl = CorePlan()
            pl.sample = b
            pl.order = order
            pl.piece_start = pc_start[sel]
            pl.piece_len = pc_len[sel]
            pl.piece_rank = pc_rank[sel]
            plans.append(pl)

    nbuckets = max(-(-len(pl.piece_len) // 128) for pl in plans)
    bl = []
    for k in range(nbuckets):
        L = 2
        for pl in plans:
            if len(pl.piece_len) > k * 128:
                L = max(L, _pow2ceil(pl.piece_len[k * 128]))
        bl.append(L)

    big = [k for k in range(nbuckets) if bl[k] >= LCAP]
    small = [k for k in range(nbuckets) if bl[k] < LCAP]

    sched = []
    # small classes first (tiny transfers warm the DVE), ascending L,
    # grouped by equal L
    for k in sorted(small, key=lambda k: bl[k]):
        if (sched and sched[-1][2] == bl[k]
                and sched[-1][0] + sched[-1][1] == k and sched[-1][1] < 4):
            sched[-1] = (sched[-1][0], sched[-1][1] + 1, bl[k])
        else:
            sched.append((k, 1, bl[k]))
    # big groups: single-bucket head (lands first, starts the DVE fold
    # train early), big middle, single-bucket tail
    nbig = len(big)
    caps = [1, 2, 3] + [4] * max(0, (nbig - 7) // 4 + 1) + [3, 1]
    k = 0
    for cap in caps:
        if k >= nbig:
            break
        G = min(cap, nbig - k)
        if nbig - (k + G) == 0 and cap != 1 and G > 1:
            G -= 1          # always leave a single-bucket final group
        sched.append((big[k], G, LCAP))
        k += G
    while k < nbig:
        sched.append((big[k], 1, LCAP))
        k += 1
    return plans, host_singles, tuple(sched)


def _build_table(pl, feats16_b, sched):
    """Pack one core's pieces into the [128, sum(G*L*64)] fp16 table."""
    totc = sum(G * L * 64 for _, G, L in sched)
    table = np.zeros((128, totc), np.float16)
    off = 0
    np_pieces = len(pl.piece_len)
    for k0, G, L in sched:
        for b in range(G):
            lo = (k0 + b) * 128
            hi = min(lo + 128, np_pieces)
            if hi > lo:
                lens = pl.piece_len[lo:hi]
                starts = pl.piece_start[lo:hi]
                p_ids = np.repeat(np.arange(hi - lo), lens)
                j_ids = np.arange(len(p_ids)) - np.repeat(
                    np.cumsum(lens) - lens, lens)
                pts = pl.order[np.repeat(starts, lens) + j_ids]
                view = table[:, off + b * L * 64: off + (b + 1) * L * 64
                             ].reshape(128, L, 64)
                view[p_ids, j_ids, :] = feats16_b[pts]
        off += G * L * 64
    return table


# ---------------- device program ----------------
def _build_kernel(sched):
    import concourse.bass as bass
    import concourse.bacc as bacc
    import concourse.mybir as mybir
    import concourse.tile as tile
    from contextlib import ExitStack

    F16 = mybir.dt.float16
    totc = sum(G * L * 64 for _, G, L in sched)
    nbt = sum(G for _, G, L in sched)

    nc = bacc.Bacc()
    table = nc.dram_tensor("table", [128, totc], F16, kind="ExternalInput")
    out = nc.dram_tensor("out", [128, nbt * 64], F16, kind="ExternalOutput")

    with tile.TileContext(nc) as tc, ExitStack() as ctx:
        pool = ctx.enter_context(tc.tile_pool(name="bkt", bufs=1))
        stp = ctx.enter_context(tc.tile_pool(name="stage", bufs=1))
        stage = stp.tile([128, nbt * 64], F16)

        # leading small-L groups share one tile + one DMA issued from the
        # idle tensor engine's ring, so it streams in parallel with the
        # sync-ring big-group DMAs instead of serializing behind their
        # ~0.6 us-per-instruction descriptor writes
        nsmall = 0
        while nsmall < len(sched) and sched[nsmall][2] < LCAP:
            nsmall += 1
        small_cols = sum(G * L * 64 for _, G, L in sched[:nsmall])

        tiles = []
        off = 0
        if nsmall:
            smt = pool.tile([128, small_cols], F16, tag="smalls")
            nc.scalar.dma_start(smt[:], table[:, 0:small_cols])
            soff = 0
            for _, G, L in sched[:nsmall]:
                tiles.append(smt[:, soff:soff + G * L * 64])
                soff += G * L * 64
            off = small_cols
        for g, (_, G, L) in enumerate(sched):
            if g < nsmall:
                continue
            t = pool.tile([128, G * L * 64], F16, tag=f"g{g}")
            nc.sync.dma_start(t[:], table[:, off:off + G * L * 64])
            tiles.append(t[:])
            off += G * L * 64

        ngroups = len(sched)
        cut1 = max(1, int(ngroups * 0.5))
        cut2 = max(cut1 + 1, int(ngroups * 0.85))
        cut3 = max(cut2 + 1, ngroups - 1)
        boffs = []
        boff = 0
        for _, G, L in sched:
            boffs.append(boff)
            boff += G

        def fold(g):
            _, G, L = sched[g]
            b0 = boffs[g]
            v = tiles[g].rearrange("p (b l c) -> p b l c", l=L, c=64)
            st = stage[:, b0 * 64:(b0 + G) * 64].rearrange(
                "p (b o c) -> p b o c", o=1, c=64)
            cur = L
            while cur > 2:
                h = cur // 2
                nc.vector.tensor_tensor(
                    v[:, :, 0:h, :], v[:, :, 0:h, :], v[:, :, h:cur, :],
                    mybir.AluOpType.add)
                cur = h
            nc.vector.tensor_tensor(
                st, v[:, :, 0:1, :], v[:, :, 1:2, :], mybir.AluOpType.add)

        # the first big group (single bucket, first on the sync ring)
        # lands before the smalls tile: fold it first
        order = ([nsmall] if ngroups > nsmall else []) \
            + list(range(nsmall)) + list(range(nsmall + 1, ngroups))
        done = 0
        out_lo = 0
        emitted = set()
        for g in order:
            fold(g)
            emitted.add(g)
            done += 1
            if done in (cut1, cut2, cut3, ngroups):
                hi = 0
                while hi < ngroups and hi in emitted:
                    hi += 1
                hi_off = boffs[hi - 1] + sched[hi - 1][1] if hi else 0
                if hi and hi_off > out_lo:
                    nc.scalar.dma_start(
                        out[:, out_lo * 64:hi_off * 64],
                        stage[:, out_lo * 64:hi_off * 64])
                    out_lo = hi_off
    nc.finalize()
    return nc


# ---------------- entry point ----------------
def kernel(image_feature, post_trans, post_rots, intrinsics, extrinsics,
           frustum, bev_res, bev_start_pos):
    from concourse.bass_utils import run_bass_kernel_spmd
    import os

    rank, mask = _compute_ranks(frustum, post_trans, post_rots, intrinsics,
                                extrinsics, bev_res, bev_start_pos)
    feats16 = np.asarray(image_feature, np.float32).reshape(
        B, NP_SAMPLE, C).astype(np.float16)
    plans, host_singles, sched = _plan_cores(rank, mask)

    in_maps = [{"table": _build_table(pl, feats16[pl.sample], sched)}
               for pl in plans]

    if sched not in _compiled:
        _compiled[sched] = _build_kernel(sched)
    nc = _compiled[sched]

    trace = bool(int(os.environ.get("BEV_TRACE", "0")))
    res = run_bass_kernel_spmd(nc, in_maps, core_ids=list(range(NCORES)),
                               trace=trace,
                               trace_cores=[0] if trace else None)
    if trace and res.exec_time_ns is not None:
        print(f"HW exec time: {res.exec_time_ns} ns")
        kernel.last_exec_time_ns = res.exec_time_ns
        kernel.last_results = res

    nbt = sum(G for _, G, L in sched)
    bucket_seq = []
    for k0, G, L in sched:
        bucket_seq.extend(range(k0, k0 + G))
    keys = []
    rows = []
    for k, pl in enumerate(plans):
        o = res.results[k]["out"]                      # [128, nbt*64] f16
        r = o.reshape(128, nbt, 64).swapaxes(0, 1)     # [slot, p, 64]
        npieces = len(pl.piece_len)
        for slot, bk in enumerate(bucket_seq):
            lo = bk * 128
            hi = min(lo + 128, npieces)
            if hi > lo:
                keys.append(pl.sample * NBINS
                            + pl.piece_rank[lo:hi].astype(np.int64))
                rows.append(r[slot, :hi - lo])
    for b, pts, rks in host_singles:
        if len(pts):
            keys.append(b * NBINS + rks.astype(np.int64))
            rows.append(feats16[b][pts])
    keys = np.concatenate(keys)
    rows = np.concatenate(rows).astype(np.float32)
    srt = np.argsort(keys, kind='stable')
    ks = keys[srt]
    bounds = np.r_[0, np.nonzero(np.diff(ks))[0] + 1]
    sums = np.add.reduceat(rows[srt], bounds, axis=0)
    grid = np.zeros((B * NBINS, C), np.float32)
    grid[ks[bounds]] = sums
    return np.ascontiguousarray(
        grid.reshape(B, X, Y, C).transpose(0, 3, 1, 2))
